# revision 40
# baseline (speedup 1.0000x reference)
"""AnomalyAttention (causal multi-head attention) on 8 TRN2 NeuronCores.

Problem: B=2, C=4, H=8, L=1024, E=64  ->  64 independent heads.
Sharding: 8 heads per core (data parallel over B*C*H), no collectives.

Per-core pipeline, software-pipelined at S^T pair-tile granularity: the
causal-valid 4608 S^T columns per head are packed into nine 512-col bins;
a PSUM pair-tile [128, 1024] holds one bin for BOTH heads of a head pair
(separate PSUM banks), 36 tiles per core:
  S^T[k, q] = sum_e K[k, e] Q[q, e]   TensorE, the two heads' 64-row matmuls
                                      run CONCURRENTLY via tile_position row
                                      tiling (measured 2x throughput)
  P^T = exp(0.125 * S^T) -> bf16      ~72% of tiles on ScalarE (exact exp),
                                      ~28% on VectorE via a one-instruction
                                      Schraudolph bitcast approximation
  diagonal 0/1 tri masks              VectorE, strided over both heads
  [O | r][q,:] = sum_k P^T[k,q] [V|1] TensorE, PSUM accumulate
  out[q, e] = O[q, e] / r[q]          VectorE reciprocal + bcast multiply

The exp window (~1.0us) is the pacing unit: during window i the PE fills
pair-tile i+1 (3 PSUM slots ping-pong) and drains a batch of AV matmuls of
the previous head pair. Input DMAs are staged: pair 0 first, pair 1
released by gate ops at early-window time, pairs 2/3 hardware-gated by
reusing pairs 0/1's SBUF tiles (bufs=1 tag sharing). Output is bf16,
upcast on the host.

Inputs are pre-transposed / bf16-cast / padded on the host as part of
sharding. Output DRAM layout [h, q%128, (q//128)*64 + e].
"""

import os
import numpy as np
from ml_dtypes import bfloat16

import concourse.bass as bass
import concourse.tile as tile
from concourse import bacc, mybir
from concourse.bass_utils import run_bass_kernel_spmd

B, C, H, L, E = 2, 4, 8, 1024, 64
N_CORES = 8
HEADS = B * C * H              # 64
HPC = HEADS // N_CORES         # 8 heads per core
NPAIR = HPC // 2               # 4 head pairs per core
NCHUNK = L // 128              # 8 k-chunks of 128
SCALE = 1.0 / 8.0

# causal S^T packing: per head, the 4608 valid columns (chunk i has width
# 1024-128i) are packed into nine 512-col bins. A PSUM pair-tile [128, 1024]
# holds one bin for BOTH heads of a pair: head 0 at cols 0:512 (bank 0),
# head 1 at 512:1024 (bank 1) — bank-pure so the two heads' S matmuls can run
# CONCURRENTLY via tile_position row tiling without sharing a PSUM bank.
# Each bin: list of (chunk, col_start_in_chunk, width, col_offset_in_bin).
S_BINS = [
    [(0, 0, 512, 0)],
    [(0, 512, 512, 0)],
    [(1, 0, 512, 0)],
    [(1, 512, 384, 0), (3, 0, 128, 384)],
    [(3, 128, 512, 0)],
    [(2, 0, 512, 0)],
    [(2, 512, 256, 0), (6, 0, 256, 256)],
    [(4, 0, 512, 0)],
    [(5, 0, 384, 0), (7, 0, 128, 384)],
]
# bin -> col offsets of 128-wide diagonal blocks needing the 0/1 tri mask
MASK_OFFS = {0: [0], 2: [0], 3: [384], 5: [0], 6: [256], 7: [0], 8: [0, 384]}

# bins whose exp runs on VectorE via the Schraudolph bitcast trick (one
# tensor_scalar: bf16_bits = int16(x * SCHRAUD_A + SCHRAUD_B)); 10/36 bins
# keeps the extra relative error ~0.6% while offloading ~28% of the exp
# work from ScalarE.  Single-diagonal tri masks run on GpSimd (otherwise
# idle), the rest on VectorE.
DVE_BINS = frozenset({1, 4})
DVE_BINS_EXTRA = frozenset({7})        # on pairs 0 and 1 only
GPS_MASK_BINS = frozenset(
    int(x) for x in os.environ.get("ATTN_GPS_MASKS", "").split(",") if x)
SCHRAUD_A = SCALE * np.log2(np.e) * 128.0
SCHRAUD_B = 127.0 * 128.0 - 6.25

# SBUF layout of the per-pair qk data, three tiles per pair so transfers can
# be released progressively (dram col order: k_c0 | q | k_c1..c7):
#   tile Aa [128, 640]: cols 0:128 = K^T chunk 0; 128:640 = Q^T q 0:512
#   tile Ab [128, 512]: Q^T q 512:1024
#   tile B  [128, 896]: K^T chunks 1..7
QAA_W, QAB_W, QKB_W = 640, 512, 896


def _q_segs(qlo, w):
    """Split q range [qlo, qlo+w) at 512 -> list of (tile_idx, col, width).
    tile_idx: 0 = Aa (col 128+q), 1 = Ab (col q-512)."""
    segs = []
    if qlo < 512:
        wa = min(w, 512 - qlo)
        segs.append((0, 128 + qlo, wa))
        qlo, w = qlo + wa, w - wa
    if w > 0:
        segs.append((1, qlo - 512, w))
    return segs

WARM_MMS = int(os.environ.get("ATTN_WARM_MMS", "6"))
KEEPALIVE = int(os.environ.get("ATTN_KEEPALIVE", "1"))
AV_BATCH = int(os.environ.get("ATTN_AV_BATCH", "8"))

LAST_RESULTS = None


def _bank_splits(c0, c1):
    """Split tile-column range [c0, c1) at 512-aligned boundaries."""
    cuts = [c0]
    nb = (c0 // 512 + 1) * 512
    while nb < c1:
        cuts.append(nb)
        nb += 512
    cuts.append(c1)
    return list(zip(cuts[:-1], cuts[1:]))


def _build_causal():
    nc = bacc.Bacc("TRN2", target_bir_lowering=False, debug=False,
                   num_devices=N_CORES)
    bf = mybir.dt.bfloat16
    f32 = mybir.dt.float32

    # qk[p]: [128, 2048] = [K^T pair-stack | Q^T pair-stack]
    qk = nc.dram_tensor("qk", [NPAIR, 128, 2 * L], bf, kind="ExternalInput").ap()
    # vo[p]: [128, 1040] = head 2p cols 0:520, head 2p+1 cols 520:1040
    vo = nc.dram_tensor("vo", [NPAIR, 128, 2 * NCHUNK * 65], bf,
                        kind="ExternalInput").ap()
    tri = nc.dram_tensor("tri", [128, 128], bf, kind="ExternalInput").ap()
    out = nc.dram_tensor("out", [HPC, 128, 512], bf, kind="ExternalOutput").ap()

    with tile.TileContext(nc) as tc:
        with (
            tc.tile_pool(name="consts", bufs=1) as consts,
            tc.tile_pool(name="pT", bufs=20) as pTpool,
            tc.tile_pool(name="psumS", bufs=3, space="PSUM") as psumS,
            tc.tile_pool(name="psumO", bufs=2, space="PSUM") as psumO,
            tc.tile_pool(name="outsb", bufs=2) as outsb,
            tc.tile_pool(name="rinvp", bufs=4) as rinvp,
        ):
            # ---- ACT exp-table warm (emitted first so the table load
            # is off the critical path) ----
            warm = consts.tile([128, 8], f32, tag="warm")
            nc.vector.memset(warm, 0.0)
            warm2 = consts.tile([128, 8], f32, tag="warm2")
            nc.scalar.activation(warm2, warm, mybir.ActivationFunctionType.Exp)
            wsrc = consts.tile([128, 260], bf, tag="wsrc")
            nc.vector.memset(wsrc, 0.0)

            # ---- staged input DMAs ----
            # Three qk tiles per pair; pairs 2/3 reuse pairs 0/1's SBUF tiles
            # (bufs=1 tag sharing), so their transfers are hardware-gated
            # until the earlier pair's last matmul read. Pair 1's transfers
            # are gated on tiny reader ops that execute at early-window time
            # on the vector queue, so pair 0's transfers get the DMA
            # bandwidth to themselves.
            qkAa_t = [consts.tile([128, QAA_W], bf, tag=f"qkAa{p % 2}",
                                  name=f"qkAa{p}") for p in range(NPAIR)]
            qkAb_t = [consts.tile([128, QAB_W], bf, tag=f"qkAb{p % 2}",
                                  name=f"qkAb{p}") for p in range(NPAIR)]
            qkB_t = [consts.tile([128, QKB_W], bf, tag=f"qkB{p % 2}",
                                 name=f"qkB{p}") for p in range(NPAIR)]
            vo_t = [consts.tile([128, 2 * NCHUNK * 65], bf, tag=f"vo{p % 2}",
                                name=f"vot{p}")
                    for p in range(NPAIR)]
            tri_t = consts.tile([128, 128], bf, tag="tri")
            gscr = consts.tile([128, 2], f32, tag="gscr")
            # init corners of the gated tiles (read by the gate ops)
            for t in (qkAa_t[1], qkAb_t[1], qkB_t[1], vo_t[1], vo_t[0]):
                nc.vector.memset(t[0:1, 0:2], 0.0)
            nc.sync.dma_start(out=qkAa_t[0], in_=qk[0][:, 0:QAA_W])
            nc.sync.dma_start(out=qkAb_t[0], in_=qk[0][:, QAA_W:QAA_W + QAB_W])
            nc.scalar.dma_start(out=qkB_t[0], in_=qk[0][:, QAA_W + QAB_W:2 * L])
            # vo0 is not needed until pair-0's AV (window 9): release it only
            # after the latency-critical qkAa0/qkAb0 transfers have landed
            nc.gpsimd.tensor_mul(gscr[0:1, 0:2], vo_t[0][0:1, 0:2],
                                 qkAb_t[0][0:1, 0:2])
            nc.gpsimd.dma_start(out=vo_t[0], in_=vo[0])
            nc.sync.dma_start(out=tri_t, in_=tri)
            # pairs 2/3 (naturally gated by tag reuse)
            for p in (2, 3):
                nc.sync.dma_start(out=qkAa_t[p], in_=qk[p][:, 0:QAA_W])
                nc.sync.dma_start(out=qkAb_t[p],
                                  in_=qk[p][:, QAA_W:QAA_W + QAB_W])
                nc.sync.dma_start(out=qkB_t[p],
                                  in_=qk[p][:, QAA_W + QAB_W:2 * L])
                nc.gpsimd.dma_start(out=vo_t[p], in_=vo[p])

            last_pts = {}

            def _gate(tile_t, ready_t):
                # reader op on the (idle) gpsimd queue: finishes only once
                # ready_t's DMA has landed, releasing the DMA emitted after it
                nc.gpsimd.tensor_mul(gscr[0:1, 0:2], tile_t[0:1, 0:2],
                                     ready_t[0:1, 0:2])

            def deferred_dma(i):
                if i == 1:
                    # release pair-1's inputs once ALL of pair-0's qk data
                    # (incl. the scalar-ring k_c1..c7 transfer) has landed
                    _gate(qkAa_t[1], qkB_t[0])
                    nc.sync.dma_start(out=qkAa_t[1], in_=qk[1][:, 0:QAA_W])
                    _gate(qkAb_t[1], qkB_t[0])
                    nc.sync.dma_start(out=qkAb_t[1],
                                      in_=qk[1][:, QAA_W:QAA_W + QAB_W])
                elif i == 4:
                    _gate(qkB_t[1], qkB_t[0])
                    nc.scalar.dma_start(out=qkB_t[1],
                                        in_=qk[1][:, QAA_W + QAB_W:2 * L])
                elif i == 7:
                    _gate(vo_t[1], vo_t[0])
                    nc.gpsimd.dma_start(out=vo_t[1], in_=vo[1])

            def pe_warm(n):
                for _ in range(n):
                    wps = psumO.tile([128, 325], f32, tag="psO", name="wps")
                    nc.tensor.matmul(wps[:, 0:260], lhsT=wsrc[:, 0:128],
                                     rhs=wsrc, start=True, stop=True)
            pe_warm(WARM_MMS)

            # ---- tile list: window order (pair, bin) ----
            tiles = [(p, b) for p in range(NPAIR) for b in range(len(S_BINS))]

            pmaps = {}          # head -> {(chunk, block): (pt, col)}
            obufs = {}
            done_halves = {}

            q_tiles = [qkAa_t, qkAb_t]

            def emit_fill(p, b):
                ps = psumS.tile([128, 1024], f32, tag="psS", name="psS")
                # the two heads' matmuls are emitted adjacent with explicit
                # row tile_position so they execute concurrently; each head
                # writes its own PSUM bank (cols 0:512 / 512:1024)
                for (ci, cc0, w, off) in S_BINS[b]:
                    if ci == 0:
                        klhs = qkAa_t[p]
                        kc = 0
                    else:
                        klhs = qkB_t[p]
                        kc = 128 * (ci - 1)
                    for (qt, qc, sw) in _q_segs(128 * ci + cc0, w):
                        for hl in (0, 1):
                            rows = slice(64 * hl, 64 * hl + 64)
                            nc.tensor.matmul(
                                ps[:, 512 * hl + off:512 * hl + off + sw],
                                lhsT=klhs[rows, kc:kc + 128],
                                rhs=q_tiles[qt][p][rows, qc:qc + sw],
                                start=True, stop=True,
                                tile_position=(64 * hl, 0),
                            )
                        off += sw
                return ps

            def emit_exp(p, b, ps):
                pt = pTpool.tile([128, 1024], bf, tag="pt", name="pt")
                if b in DVE_BINS or (b in DVE_BINS_EXTRA and p < 2):
                    # fast exp on VectorE: bf16 bit pattern built directly by
                    # an affine op + float->int16 convert (Schraudolph)
                    pti = pt[:, :].bitcast(mybir.dt.int16)
                    nc.vector.tensor_scalar(
                        pti, ps, SCHRAUD_A, SCHRAUD_B,
                        mybir.AluOpType.mult, mybir.AluOpType.add)
                else:
                    nc.scalar.activation(
                        pt, ps, mybir.ActivationFunctionType.Exp, scale=SCALE)
                for hl in (0, 1):
                    h = 2 * p + hl
                    pm = pmaps.setdefault(h, {})
                    for (ci, cc0, w, off) in S_BINS[b]:
                        for blk in range(w // 128):
                            pm[(ci, cc0 // 128 + blk)] = (
                                pt, 512 * hl + off + 128 * blk)
                return pt

            def emit_mask(b, pt):
                offs = MASK_OFFS.get(b)
                if not offs:
                    return
                ptap = pt[:, :]
                tap = tri_t[:, :]
                if len(offs) == 1:
                    free = [[512, 2], [1, 128]]
                    tfree = [[0, 2], [1, 128]]
                else:
                    free = [[512, 2], [offs[1] - offs[0], 2], [1, 128]]
                    tfree = [[0, 2], [0, 2], [1, 128]]
                src = bass.AP(tensor=ptap.tensor, offset=ptap.offset + offs[0],
                              ap=[ptap.ap[0]] + free)
                trib = bass.AP(tensor=tap.tensor, offset=tap.offset,
                               ap=[tap.ap[0]] + tfree)
                eng = nc.gpsimd if b in GPS_MASK_BINS else nc.vector
                eng.tensor_mul(src, src, trib)

            # ---- AV op queue ----
            # items: ("mm", fn) costing 1, ("aux", fn) costing 0
            av_q = []
            cur_po = [None]     # live psumO tile for keepalives

            def make_unit_ops(p, h, half, last_pair):
                hl = h - 2 * p
                ops = []

                def alloc_po():
                    po = psumO.tile([128, 325], f32, tag="psO", name="psO")
                    cur_po[0] = po
                    if h not in obufs:
                        obufs[h] = outsb.tile([128, 512], bf,
                                              tag=f"ob{h % 4}", name=f"ob{h}")
                    return po

                state = {}

                def first_mm():
                    state["po"] = alloc_po()

                for bi in range(4):
                    j, coff = 4 * half + bi, 65 * bi
                    ks = list(range(j + 1))
                    for idx, i2 in enumerate(ks):
                        def mm(bi=bi, j=j, coff=coff, idx=idx, i2=i2,
                               first=(bi == 0 and idx == 0)):
                            if first:
                                first_mm()
                            po = state["po"]
                            pt, cs = pmaps[h][(i2, j - i2)]
                            nc.tensor.matmul(
                                po[:, coff:coff + 65],
                                lhsT=pt[:, cs:cs + 128],
                                rhs=vo_t[p][:, 520 * hl + 65 * i2:
                                            520 * hl + 65 * i2 + 65],
                                start=(idx == 0), stop=(idx == len(ks) - 1),
                            )
                        ops.append(("mm", mm))

                def aux():
                    po = state["po"]
                    obuf = obufs[h]
                    rinv4 = rinvp.tile([128, 4], mybir.dt.float32,
                                       tag="rinv", name="rinv")
                    poap = po[:, :]
                    rsrc = bass.AP(tensor=poap.tensor, offset=poap.offset + 64,
                                   ap=[poap.ap[0], [65, 4]])
                    nc.vector.reciprocal(rinv4, rsrc)
                    o_in = bass.AP(tensor=poap.tensor, offset=poap.offset,
                                   ap=[poap.ap[0], [65, 4], [1, 64]])
                    rap = rinv4[:, :]
                    r_in = bass.AP(tensor=rap.tensor, offset=rap.offset,
                                   ap=[rap.ap[0], [1, 4], [0, 64]])
                    oap = obuf[:, :]
                    o_out = bass.AP(tensor=oap.tensor,
                                    offset=oap.offset + 256 * half,
                                    ap=[oap.ap[0], [64, 4], [1, 64]])
                    nc.vector.tensor_mul(o_out, o_in, r_in)
                    if cur_po[0] is po:
                        cur_po[0] = None
                    done_halves.setdefault(h, set()).add(half)
                    if last_pair:
                        # stream each finished 256-col slab out immediately;
                        # scalar queue is idle after the last exp while sync
                        # is backlogged with earlier triggers
                        nc.scalar.dma_start(
                            out=out[h][:, 256 * half:256 * half + 256],
                            in_=obuf[:, 256 * half:256 * half + 256])
                        if len(done_halves[h]) == 2:
                            obufs.pop(h)
                    elif len(done_halves[h]) == 2:
                        nc.sync.dma_start(out=out[h], in_=obufs.pop(h))
                ops.append(("aux", aux))
                return ops

            def append_pair_av(p, last_pair):
                order = [(2 * p, 0), (2 * p + 1, 0), (2 * p, 1), (2 * p + 1, 1)]
                for h, half in order:
                    av_q.extend(make_unit_ops(p, h, half, last_pair))

            def keepalive():
                po = cur_po[0]
                if KEEPALIVE and po is not None:
                    # accumulate 0 into scratch cols; start=False so the
                    # bank's live has_written state is untouched
                    nc.tensor.matmul(po[:, 260:325], lhsT=wsrc[:, 0:128],
                                     rhs=wsrc[:, 0:65], start=False, stop=False,
                                     skip_group_check=True)

            def drain(n_mms):
                got = 0
                since_ka = 0
                while av_q and got < n_mms:
                    kind, fn = av_q.pop(0)
                    fn()
                    if kind == "mm":
                        got += 1
                        since_ka += 1
                        if since_ka >= 6:
                            keepalive()
                            since_ka = 0
                return got

            # ---- main pipeline ----
            nb = len(S_BINS)
            pl = NPAIR - 1
            for i, (p, b) in enumerate(tiles):
                deferred_dma(i)
                if p >= 1 and (i == p * nb):
                    append_pair_av(p - 1, last_pair=False)
                if i == (NPAIR - 1) * nb + 7:
                    # last pair: half-0 AV needs only chunks 0..3 (bins 0..6,
                    # all emitted) — overlap it with the final two windows
                    for h in (2 * pl, 2 * pl + 1):
                        av_q.extend(make_unit_ops(pl, h, 0, last_pair=True))
                ps = emit_fill(p, b)
                pt = emit_exp(p, b, ps)
                last_pts[i] = pt
                emit_mask(b, pt)
                drain(AV_BATCH if i < (NPAIR - 1) * nb + 7 else AV_BATCH + 6)
            # only the last pair's half-1 AV remains after the final exp
            for h, half in [(2 * pl, 1), (2 * pl + 1, 1)]:
                av_q.extend(make_unit_ops(pl, h, half, last_pair=True))
            while av_q:
                kind, fn = av_q.pop(0)
                fn()
    nc.compile()
    return nc


def _build_noncausal():
    """Fallback path (reference mask that is not the causal triangle)."""
    nc = bacc.Bacc("TRN2", target_bir_lowering=False, debug=False,
                   num_devices=N_CORES)
    bf = mybir.dt.bfloat16
    f32 = mybir.dt.float32

    qT = nc.dram_tensor("qT", [NPAIR, 128, L], bf, kind="ExternalInput").ap()
    kT = nc.dram_tensor("kT", [NPAIR, 128, L], bf, kind="ExternalInput").ap()
    vo = nc.dram_tensor("vo", [HPC, 128, NCHUNK * 65], bf, kind="ExternalInput").ap()
    msk = nc.dram_tensor("msk", [NCHUNK, 128, L], bf, kind="ExternalInput").ap()
    out = nc.dram_tensor("out", [HPC, 128, 512], bf, kind="ExternalOutput").ap()

    with tile.TileContext(nc) as tc:
        with (
            tc.tile_pool(name="consts", bufs=1) as consts,
            tc.tile_pool(name="pT", bufs=32) as pTpool,
            tc.tile_pool(name="psumS", bufs=2, space="PSUM") as psumS,
            tc.tile_pool(name="psumO", bufs=2, space="PSUM") as psumO,
            tc.tile_pool(name="outsb", bufs=2) as outsb,
            tc.tile_pool(name="rinvp", bufs=4) as rinvp,
        ):
            qT_t = [consts.tile([128, L], bf, tag=f"qT{p}") for p in range(NPAIR)]
            kT_t = [consts.tile([128, L], bf, tag=f"kT{p}") for p in range(NPAIR)]
            vo_t = [consts.tile([128, NCHUNK * 65], bf, tag=f"vo{h}")
                    for h in range(HPC)]
            nc.sync.dma_start(out=kT_t[0], in_=kT[0])
            nc.sync.dma_start(out=qT_t[0], in_=qT[0])
            for h in (0, 1):
                nc.gpsimd.dma_start(out=vo_t[h], in_=vo[h])
            for p in (1, 2, 3):
                nc.sync.dma_start(out=kT_t[p], in_=kT[p])
                nc.sync.dma_start(out=qT_t[p], in_=qT[p])
            for h in range(2, HPC):
                nc.gpsimd.dma_start(out=vo_t[h], in_=vo[h])
            msk_t = []
            for c in range(NCHUNK):
                t = consts.tile([128, L], bf, tag=f"msk{c}")
                nc.gpsimd.dma_start(out=t, in_=msk[c])
                msk_t.append(t)

            pmaps = {}
            # simple per-chunk pipeline (correctness-focused fallback)
            for p in range(NPAIR):
                for hl, rows in ((0, slice(0, 64)), (1, slice(64, 128))):
                    h = 2 * p + hl
                    pmaps[h] = [None] * NCHUNK
                    for ci in range(NCHUNK):
                        ps = psumS.tile([128, 1024], f32, tag="psS", name="psS")
                        for s0 in range(0, L, 512):
                            nc.tensor.matmul(
                                ps[:, s0:s0 + 512],
                                lhsT=kT_t[p][rows, 128 * ci:128 * ci + 128],
                                rhs=qT_t[p][rows, s0:s0 + 512],
                                start=True, stop=True,
                            )
                        pt = pTpool.tile([128, 1024], bf, tag="pt", name="pt")
                        nc.scalar.activation(
                            pt, ps, mybir.ActivationFunctionType.Exp,
                            scale=SCALE)
                        nc.vector.tensor_mul(pt, pt, msk_t[ci])
                        pmaps[h][ci] = pt
                    obuf = outsb.tile([128, 512], f32, tag=f"ob{h % 4}",
                                      name=f"ob{h}")
                    for half in (0, 1):
                        po = psumO.tile([128, 325], f32, tag="psO", name="psO")
                        for bi in range(4):
                            j, coff = 4 * half + bi, 65 * bi
                            for idx, i2 in enumerate(range(NCHUNK)):
                                pt = pmaps[h][i2]
                                nc.tensor.matmul(
                                    po[:, coff:coff + 65],
                                    lhsT=pt[:, 128 * j:128 * j + 128],
                                    rhs=vo_t[h][:, 65 * i2:65 * i2 + 65],
                                    start=(idx == 0), stop=(idx == NCHUNK - 1),
                                )
                        rinv4 = rinvp.tile([128, 4], f32, tag="rinv", name="rinv")
                        poap = po[:, :]
                        rsrc = bass.AP(tensor=poap.tensor,
                                       offset=poap.offset + 64,
                                       ap=[poap.ap[0], [65, 4]])
                        nc.vector.reciprocal(rinv4, rsrc)
                        o_in = bass.AP(tensor=poap.tensor, offset=poap.offset,
                                       ap=[poap.ap[0], [65, 4], [1, 64]])
                        rap = rinv4[:, :]
                        r_in = bass.AP(tensor=rap.tensor, offset=rap.offset,
                                       ap=[rap.ap[0], [1, 4], [0, 64]])
                        oap = obuf[:, :]
                        o_out = bass.AP(tensor=oap.tensor,
                                        offset=oap.offset + 256 * half,
                                        ap=[oap.ap[0], [64, 4], [1, 64]])
                        nc.vector.tensor_mul(o_out, o_in, r_in)
                    nc.sync.dma_start(out=out[h], in_=obuf)
    nc.compile()
    return nc


_CACHE = {}


def _get_nc(causal: bool):
    if causal not in _CACHE:
        _CACHE[causal] = _build_causal() if causal else _build_noncausal()
    return _CACHE[causal]


def kernel(queries, keys, values, attn_mask):
    global LAST_RESULTS
    q = np.asarray(queries).reshape(HEADS, L, E)
    k = np.asarray(keys).reshape(HEADS, L, E)
    v = np.asarray(values).reshape(HEADS, L, E)
    mask = np.asarray(attn_mask).reshape(L, L)
    causal = bool(np.array_equal(mask, np.triu(np.ones((L, L), bool), k=1)))

    nc = _get_nc(causal)

    tri = np.triu(np.ones((128, 128), np.float32), k=0).astype(bfloat16)
    if not causal:
        m01 = np.where(mask, 0.0, 1.0).astype(np.float32).T
        msk = np.ascontiguousarray(m01).reshape(NCHUNK, 128, L).astype(bfloat16)

    in_maps = []
    for c in range(N_CORES):
        hs = slice(c * HPC, (c + 1) * HPC)
        qTm = np.ascontiguousarray(
            q[hs].transpose(0, 2, 1)).astype(bfloat16).reshape(NPAIR, 128, L)
        kTm = np.ascontiguousarray(
            k[hs].transpose(0, 2, 1)).astype(bfloat16).reshape(NPAIR, 128, L)
        vh = v[hs].astype(np.float32)
        vcat = np.concatenate(
            [vh, np.ones((HPC, L, 1), np.float32)], axis=2)  # [8, L, 65]
        vom = np.ascontiguousarray(
            vcat.reshape(HPC, NCHUNK, 128, 65).transpose(0, 2, 1, 3)
        ).astype(bfloat16).reshape(HPC, 128, NCHUNK * 65)
        if causal:
            # split layout: [K^T c0 | Q^T 0:512 | Q^T 512:1024 | K^T c1..c7]
            qkm = np.ascontiguousarray(np.concatenate(
                [kTm[:, :, 0:128], qTm[:, :, 0:512],
                 qTm[:, :, 512:1024], kTm[:, :, 128:1024]],
                axis=2))                                      # [NPAIR,128,2048]
            vop = np.ascontiguousarray(
                vom.reshape(NPAIR, 2, 128, NCHUNK * 65).transpose(0, 2, 1, 3)
            ).reshape(NPAIR, 128, 2 * NCHUNK * 65)
            im = {"qk": qkm, "vo": vop, "tri": tri}
        else:
            im = {"qT": qTm, "kT": kTm, "vo": vom, "msk": msk}
        in_maps.append(im)

    trace = bool(os.environ.get("BASS_ATTN_TRACE"))
    res = run_bass_kernel_spmd(nc, in_maps, core_ids=list(range(N_CORES)),
                               trace=trace)
    LAST_RESULTS = res
    # out[c]: [HPC, 128, 512] = [h, p, j*64+e]; q = 128*j + p
    outs = np.stack([res.results[c]["out"] for c in range(N_CORES)])
    outs = outs.reshape(N_CORES, HPC, 128, NCHUNK, E).transpose(0, 1, 3, 2, 4)
    return np.ascontiguousarray(
        outs.reshape(B, C, H, L, E)).astype(np.float32)


# revision 41
# speedup vs baseline: 1.0389x; 1.0389x over previous
"""AnomalyAttention (causal multi-head attention) on 8 TRN2 NeuronCores.

Problem: B=2, C=4, H=8, L=1024, E=64  ->  64 independent heads.
Sharding: 8 heads per core (data parallel over B*C*H), no collectives.

Per-core pipeline, software-pipelined at S^T pair-tile granularity: the
causal-valid 4608 S^T columns per head are packed into nine 512-col bins;
a PSUM pair-tile [128, 1024] holds one bin for BOTH heads of a head pair
(separate PSUM banks), 36 tiles per core:
  S^T[k, q] = sum_e K[k, e] Q[q, e]   TensorE, the two heads' 64-row matmuls
                                      run CONCURRENTLY via tile_position row
                                      tiling (measured 2x throughput)
  P^T = exp(0.125 * S^T) -> bf16      ~72% of tiles on ScalarE (exact exp),
                                      ~28% on VectorE via a one-instruction
                                      Schraudolph bitcast approximation
  diagonal 0/1 tri masks              VectorE, strided over both heads
  [O | r][q,:] = sum_k P^T[k,q] [V|1] TensorE, PSUM accumulate
  out[q, e] = O[q, e] / r[q]          VectorE reciprocal + bcast multiply

The exp window (~1.0us) is the pacing unit: during window i the PE fills
pair-tile i+1 (3 PSUM slots ping-pong) and drains a batch of AV matmuls of
the previous head pair. Input DMAs are staged: pair 0 first, pair 1
released by gate ops at early-window time, pairs 2/3 hardware-gated by
reusing pairs 0/1's SBUF tiles (bufs=1 tag sharing). Output is bf16,
upcast on the host.

Inputs are pre-transposed / bf16-cast / padded on the host as part of
sharding. Output DRAM layout [h, q%128, (q//128)*64 + e].
"""

import os
import numpy as np
from ml_dtypes import bfloat16

import concourse.bass as bass
import concourse.tile as tile
from concourse import bacc, mybir
from concourse.bass_utils import run_bass_kernel_spmd

B, C, H, L, E = 2, 4, 8, 1024, 64
N_CORES = 8
HEADS = B * C * H              # 64
HPC = HEADS // N_CORES         # 8 heads per core
NPAIR = HPC // 2               # 4 head pairs per core
NCHUNK = L // 128              # 8 k-chunks of 128
SCALE = 1.0 / 8.0

# causal S^T packing: per head, the 4608 valid columns (chunk i has width
# 1024-128i) are packed into nine 512-col bins. A PSUM pair-tile [128, 1024]
# holds one bin for BOTH heads of a pair: head 0 at cols 0:512 (bank 0),
# head 1 at 512:1024 (bank 1) — bank-pure so the two heads' S matmuls can run
# CONCURRENTLY via tile_position row tiling without sharing a PSUM bank.
# Each bin: list of (chunk, col_start_in_chunk, width, col_offset_in_bin).
S_BINS = [
    [(0, 0, 512, 0)],
    [(0, 512, 512, 0)],
    [(1, 0, 512, 0)],
    [(1, 512, 384, 0), (3, 0, 128, 384)],
    [(3, 128, 512, 0)],
    [(2, 0, 512, 0)],
    [(2, 512, 256, 0), (6, 0, 256, 256)],
    [(4, 0, 512, 0)],
    [(5, 0, 384, 0), (7, 0, 128, 384)],
]
# bin -> col offsets of 128-wide diagonal blocks needing the 0/1 tri mask
MASK_OFFS = {0: [0], 2: [0], 3: [384], 5: [0], 6: [256], 7: [0], 8: [0, 384]}

# bins whose exp runs on VectorE via the Schraudolph bitcast trick (one
# tensor_scalar: bf16_bits = int16(x * SCHRAUD_A + SCHRAUD_B)); 10/36 bins
# keeps the extra relative error ~0.6% while offloading ~28% of the exp
# work from ScalarE.  Single-diagonal tri masks run on GpSimd (otherwise
# idle), the rest on VectorE.
DVE_BINS = frozenset({1, 4})
DVE_BINS_EXTRA = frozenset({7})        # on pairs 0 and 1 only
GPS_MASK_BINS = frozenset(
    int(x) for x in os.environ.get("ATTN_GPS_MASKS", "").split(",") if x)
SCHRAUD_A = SCALE * np.log2(np.e) * 128.0
SCHRAUD_B = 127.0 * 128.0 - 6.25

# SBUF layout of the per-pair qk data, three tiles per pair so transfers can
# be released progressively (dram col order: k_c0 | q | k_c1..c7):
#   tile Aa [128, 640]: cols 0:128 = K^T chunk 0; 128:640 = Q^T q 0:512
#   tile Ab [128, 512]: Q^T q 512:1024
#   tile B  [128, 896]: K^T chunks 1..7
QAA_W, QAB_W, QKB_W = 640, 512, 896


def _q_segs(qlo, w):
    """Split q range [qlo, qlo+w) at 512 -> list of (tile_idx, col, width).
    tile_idx: 0 = Aa (col 128+q), 1 = Ab (col q-512)."""
    segs = []
    if qlo < 512:
        wa = min(w, 512 - qlo)
        segs.append((0, 128 + qlo, wa))
        qlo, w = qlo + wa, w - wa
    if w > 0:
        segs.append((1, qlo - 512, w))
    return segs

WARM_MMS = int(os.environ.get("ATTN_WARM_MMS", "6"))
KEEPALIVE = int(os.environ.get("ATTN_KEEPALIVE", "1"))
AV_BATCH = int(os.environ.get("ATTN_AV_BATCH", "8"))

LAST_RESULTS = None


def _bank_splits(c0, c1):
    """Split tile-column range [c0, c1) at 512-aligned boundaries."""
    cuts = [c0]
    nb = (c0 // 512 + 1) * 512
    while nb < c1:
        cuts.append(nb)
        nb += 512
    cuts.append(c1)
    return list(zip(cuts[:-1], cuts[1:]))


def _build_causal():
    nc = bacc.Bacc("TRN2", target_bir_lowering=False, debug=False,
                   num_devices=N_CORES)
    bf = mybir.dt.bfloat16
    f32 = mybir.dt.float32

    # qk[p]: [128, 2048] = [K^T pair-stack | Q^T pair-stack]
    qk = nc.dram_tensor("qk", [NPAIR, 128, 2 * L], bf, kind="ExternalInput").ap()
    # vo[p]: [128, 1040] = head 2p cols 0:520, head 2p+1 cols 520:1040
    vo = nc.dram_tensor("vo", [NPAIR, 128, 2 * NCHUNK * 65], bf,
                        kind="ExternalInput").ap()
    tri = nc.dram_tensor("tri", [128, 128], bf, kind="ExternalInput").ap()
    out = nc.dram_tensor("out", [HPC, 128, 512], bf, kind="ExternalOutput").ap()

    with tile.TileContext(nc) as tc:
        with (
            tc.tile_pool(name="consts", bufs=1) as consts,
            tc.tile_pool(name="pT", bufs=20) as pTpool,
            tc.tile_pool(name="psumS", bufs=3, space="PSUM") as psumS,
            tc.tile_pool(name="psumO", bufs=2, space="PSUM") as psumO,
            tc.tile_pool(name="outsb", bufs=2) as outsb,
            tc.tile_pool(name="rinvp", bufs=4) as rinvp,
        ):
            # ---- ACT exp-table warm (emitted first so the table load
            # is off the critical path) ----
            warm = consts.tile([128, 8], f32, tag="warm")
            nc.vector.memset(warm, 0.0)
            warm2 = consts.tile([128, 8], f32, tag="warm2")
            nc.scalar.activation(warm2, warm, mybir.ActivationFunctionType.Exp)
            wsrc = consts.tile([128, 260], bf, tag="wsrc")
            nc.vector.memset(wsrc, 0.0)

            # ---- staged input DMAs ----
            # Three qk tiles per pair; pairs 2/3 reuse pairs 0/1's SBUF tiles
            # (bufs=1 tag sharing), so their transfers are hardware-gated
            # until the earlier pair's last matmul read. Pair 1's transfers
            # are gated on tiny reader ops that execute at early-window time
            # on the vector queue, so pair 0's transfers get the DMA
            # bandwidth to themselves.
            qkAa_t = [consts.tile([128, QAA_W], bf, tag=f"qkAa{p % 2}",
                                  name=f"qkAa{p}") for p in range(NPAIR)]
            qkAb_t = [consts.tile([128, QAB_W], bf, tag=f"qkAb{p % 2}",
                                  name=f"qkAb{p}") for p in range(NPAIR)]
            qkB_t = [consts.tile([128, QKB_W], bf, tag=f"qkB{p % 2}",
                                 name=f"qkB{p}") for p in range(NPAIR)]
            vo_t = [consts.tile([128, 2 * NCHUNK * 65], bf, tag=f"vo{p % 2}",
                                name=f"vot{p}")
                    for p in range(NPAIR)]
            tri_t = consts.tile([128, 128], bf, tag="tri")
            gscr = consts.tile([128, 2], f32, tag="gscr")
            # init corners of the gated tiles (read by the gate ops)
            for t in (qkAa_t[1], qkAb_t[1], qkB_t[1], vo_t[1], vo_t[0]):
                nc.vector.memset(t[0:1, 0:2], 0.0)
            nc.sync.dma_start(out=qkAa_t[0], in_=qk[0][:, 0:QAA_W])
            nc.sync.dma_start(out=qkAb_t[0], in_=qk[0][:, QAA_W:QAA_W + QAB_W])
            nc.scalar.dma_start(out=qkB_t[0], in_=qk[0][:, QAA_W + QAB_W:2 * L])
            nc.gpsimd.dma_start(out=vo_t[0], in_=vo[0])
            nc.sync.dma_start(out=tri_t, in_=tri)
            # pairs 2/3 (naturally gated by tag reuse)
            for p in (2, 3):
                nc.sync.dma_start(out=qkAa_t[p], in_=qk[p][:, 0:QAA_W])
                nc.sync.dma_start(out=qkAb_t[p],
                                  in_=qk[p][:, QAA_W:QAA_W + QAB_W])
                nc.sync.dma_start(out=qkB_t[p],
                                  in_=qk[p][:, QAA_W + QAB_W:2 * L])
                nc.gpsimd.dma_start(out=vo_t[p], in_=vo[p])

            last_pts = {}

            def _gate(tile_t, ready_t):
                # reader op on the (idle) gpsimd queue: finishes only once
                # ready_t's DMA has landed, releasing the DMA emitted after it
                nc.gpsimd.tensor_mul(gscr[0:1, 0:2], tile_t[0:1, 0:2],
                                     ready_t[0:1, 0:2])

            def deferred_dma(i):
                if i == 1:
                    # release pair-1's inputs once ALL of pair-0's qk data
                    # (incl. the scalar-ring k_c1..c7 transfer) has landed
                    _gate(qkAa_t[1], qkB_t[0])
                    nc.sync.dma_start(out=qkAa_t[1], in_=qk[1][:, 0:QAA_W])
                    _gate(qkAb_t[1], qkB_t[0])
                    nc.sync.dma_start(out=qkAb_t[1],
                                      in_=qk[1][:, QAA_W:QAA_W + QAB_W])
                elif i == 4:
                    _gate(qkB_t[1], qkB_t[0])
                    nc.scalar.dma_start(out=qkB_t[1],
                                        in_=qk[1][:, QAA_W + QAB_W:2 * L])
                elif i == 7:
                    _gate(vo_t[1], vo_t[0])
                    nc.gpsimd.dma_start(out=vo_t[1], in_=vo[1])

            def pe_warm(n):
                for _ in range(n):
                    wps = psumO.tile([128, 325], f32, tag="psO", name="wps")
                    nc.tensor.matmul(wps[:, 0:260], lhsT=wsrc[:, 0:128],
                                     rhs=wsrc, start=True, stop=True)
            pe_warm(WARM_MMS)

            # ---- tile list: window order (pair, bin) ----
            tiles = [(p, b) for p in range(NPAIR) for b in range(len(S_BINS))]

            pmaps = {}          # head -> {(chunk, block): (pt, col)}
            obufs = {}
            done_halves = {}

            q_tiles = [qkAa_t, qkAb_t]

            def emit_fill(p, b):
                ps = psumS.tile([128, 1024], f32, tag="psS", name="psS")
                # the two heads' matmuls are emitted adjacent with explicit
                # row tile_position so they execute concurrently; each head
                # writes its own PSUM bank (cols 0:512 / 512:1024)
                for (ci, cc0, w, off) in S_BINS[b]:
                    if ci == 0:
                        klhs = qkAa_t[p]
                        kc = 0
                    else:
                        klhs = qkB_t[p]
                        kc = 128 * (ci - 1)
                    for (qt, qc, sw) in _q_segs(128 * ci + cc0, w):
                        for hl in (0, 1):
                            rows = slice(64 * hl, 64 * hl + 64)
                            nc.tensor.matmul(
                                ps[:, 512 * hl + off:512 * hl + off + sw],
                                lhsT=klhs[rows, kc:kc + 128],
                                rhs=q_tiles[qt][p][rows, qc:qc + sw],
                                start=True, stop=True,
                                tile_position=(64 * hl, 0),
                            )
                        off += sw
                return ps

            def emit_exp(p, b, ps):
                pt = pTpool.tile([128, 1024], bf, tag="pt", name="pt")
                if b in DVE_BINS or (b in DVE_BINS_EXTRA and p < 2):
                    # fast exp on VectorE: bf16 bit pattern built directly by
                    # an affine op + float->int16 convert (Schraudolph)
                    pti = pt[:, :].bitcast(mybir.dt.int16)
                    nc.vector.tensor_scalar(
                        pti, ps, SCHRAUD_A, SCHRAUD_B,
                        mybir.AluOpType.mult, mybir.AluOpType.add)
                else:
                    nc.scalar.activation(
                        pt, ps, mybir.ActivationFunctionType.Exp, scale=SCALE)
                for hl in (0, 1):
                    h = 2 * p + hl
                    pm = pmaps.setdefault(h, {})
                    for (ci, cc0, w, off) in S_BINS[b]:
                        for blk in range(w // 128):
                            pm[(ci, cc0 // 128 + blk)] = (
                                pt, 512 * hl + off + 128 * blk)
                return pt

            def emit_mask(b, pt):
                offs = MASK_OFFS.get(b)
                if not offs:
                    return
                ptap = pt[:, :]
                tap = tri_t[:, :]
                if len(offs) == 1:
                    free = [[512, 2], [1, 128]]
                    tfree = [[0, 2], [1, 128]]
                else:
                    free = [[512, 2], [offs[1] - offs[0], 2], [1, 128]]
                    tfree = [[0, 2], [0, 2], [1, 128]]
                src = bass.AP(tensor=ptap.tensor, offset=ptap.offset + offs[0],
                              ap=[ptap.ap[0]] + free)
                trib = bass.AP(tensor=tap.tensor, offset=tap.offset,
                               ap=[tap.ap[0]] + tfree)
                eng = nc.gpsimd if b in GPS_MASK_BINS else nc.vector
                eng.tensor_mul(src, src, trib)

            # ---- AV op queue ----
            # items: ("mm", fn) costing 1, ("aux", fn) costing 0
            av_q = []
            cur_po = [None]     # live psumO tile for keepalives

            def make_unit_ops(p, h, half, last_pair):
                hl = h - 2 * p
                ops = []

                def alloc_po():
                    po = psumO.tile([128, 325], f32, tag="psO", name="psO")
                    cur_po[0] = po
                    if h not in obufs:
                        obufs[h] = outsb.tile([128, 512], bf,
                                              tag=f"ob{h % 4}", name=f"ob{h}")
                    return po

                state = {}

                def first_mm():
                    state["po"] = alloc_po()

                for bi in range(4):
                    j, coff = 4 * half + bi, 65 * bi
                    ks = list(range(j + 1))
                    for idx, i2 in enumerate(ks):
                        def mm(bi=bi, j=j, coff=coff, idx=idx, i2=i2,
                               first=(bi == 0 and idx == 0)):
                            if first:
                                first_mm()
                            po = state["po"]
                            pt, cs = pmaps[h][(i2, j - i2)]
                            nc.tensor.matmul(
                                po[:, coff:coff + 65],
                                lhsT=pt[:, cs:cs + 128],
                                rhs=vo_t[p][:, 520 * hl + 65 * i2:
                                            520 * hl + 65 * i2 + 65],
                                start=(idx == 0), stop=(idx == len(ks) - 1),
                            )
                        ops.append(("mm", mm))

                def aux():
                    po = state["po"]
                    obuf = obufs[h]
                    rinv4 = rinvp.tile([128, 4], mybir.dt.float32,
                                       tag="rinv", name="rinv")
                    poap = po[:, :]
                    rsrc = bass.AP(tensor=poap.tensor, offset=poap.offset + 64,
                                   ap=[poap.ap[0], [65, 4]])
                    nc.vector.reciprocal(rinv4, rsrc)
                    o_in = bass.AP(tensor=poap.tensor, offset=poap.offset,
                                   ap=[poap.ap[0], [65, 4], [1, 64]])
                    rap = rinv4[:, :]
                    r_in = bass.AP(tensor=rap.tensor, offset=rap.offset,
                                   ap=[rap.ap[0], [1, 4], [0, 64]])
                    oap = obuf[:, :]
                    o_out = bass.AP(tensor=oap.tensor,
                                    offset=oap.offset + 256 * half,
                                    ap=[oap.ap[0], [64, 4], [1, 64]])
                    nc.vector.tensor_mul(o_out, o_in, r_in)
                    if cur_po[0] is po:
                        cur_po[0] = None
                    done_halves.setdefault(h, set()).add(half)
                    if last_pair:
                        # stream each finished 256-col slab out immediately;
                        # scalar queue is idle after the last exp while sync
                        # is backlogged with earlier triggers
                        nc.scalar.dma_start(
                            out=out[h][:, 256 * half:256 * half + 256],
                            in_=obuf[:, 256 * half:256 * half + 256])
                        if len(done_halves[h]) == 2:
                            obufs.pop(h)
                    elif len(done_halves[h]) == 2:
                        nc.sync.dma_start(out=out[h], in_=obufs.pop(h))
                ops.append(("aux", aux))
                return ops

            def append_pair_av(p, last_pair):
                order = [(2 * p, 0), (2 * p + 1, 0), (2 * p, 1), (2 * p + 1, 1)]
                for h, half in order:
                    av_q.extend(make_unit_ops(p, h, half, last_pair))

            def keepalive():
                po = cur_po[0]
                if KEEPALIVE and po is not None:
                    # accumulate 0 into scratch cols; start=False so the
                    # bank's live has_written state is untouched
                    nc.tensor.matmul(po[:, 260:325], lhsT=wsrc[:, 0:128],
                                     rhs=wsrc[:, 0:65], start=False, stop=False,
                                     skip_group_check=True)

            def drain(n_mms):
                got = 0
                since_ka = 0
                while av_q and got < n_mms:
                    kind, fn = av_q.pop(0)
                    fn()
                    if kind == "mm":
                        got += 1
                        since_ka += 1
                        if since_ka >= 6:
                            keepalive()
                            since_ka = 0
                return got

            # ---- main pipeline ----
            nb = len(S_BINS)
            pl = NPAIR - 1
            for i, (p, b) in enumerate(tiles):
                deferred_dma(i)
                if p >= 1 and (i == p * nb):
                    append_pair_av(p - 1, last_pair=False)
                if i == (NPAIR - 1) * nb + 7:
                    # last pair: half-0 AV needs only chunks 0..3 (bins 0..6,
                    # all emitted) — overlap it with the final two windows
                    for h in (2 * pl, 2 * pl + 1):
                        av_q.extend(make_unit_ops(pl, h, 0, last_pair=True))
                ps = emit_fill(p, b)
                pt = emit_exp(p, b, ps)
                last_pts[i] = pt
                emit_mask(b, pt)
                drain(AV_BATCH if i < (NPAIR - 1) * nb + 7 else AV_BATCH + 6)
            # only the last pair's half-1 AV remains after the final exp
            for h, half in [(2 * pl, 1), (2 * pl + 1, 1)]:
                av_q.extend(make_unit_ops(pl, h, half, last_pair=True))
            while av_q:
                kind, fn = av_q.pop(0)
                fn()
    nc.compile()
    return nc


def _build_noncausal():
    """Fallback path (reference mask that is not the causal triangle)."""
    nc = bacc.Bacc("TRN2", target_bir_lowering=False, debug=False,
                   num_devices=N_CORES)
    bf = mybir.dt.bfloat16
    f32 = mybir.dt.float32

    qT = nc.dram_tensor("qT", [NPAIR, 128, L], bf, kind="ExternalInput").ap()
    kT = nc.dram_tensor("kT", [NPAIR, 128, L], bf, kind="ExternalInput").ap()
    vo = nc.dram_tensor("vo", [HPC, 128, NCHUNK * 65], bf, kind="ExternalInput").ap()
    msk = nc.dram_tensor("msk", [NCHUNK, 128, L], bf, kind="ExternalInput").ap()
    out = nc.dram_tensor("out", [HPC, 128, 512], bf, kind="ExternalOutput").ap()

    with tile.TileContext(nc) as tc:
        with (
            tc.tile_pool(name="consts", bufs=1) as consts,
            tc.tile_pool(name="pT", bufs=32) as pTpool,
            tc.tile_pool(name="psumS", bufs=2, space="PSUM") as psumS,
            tc.tile_pool(name="psumO", bufs=2, space="PSUM") as psumO,
            tc.tile_pool(name="outsb", bufs=2) as outsb,
            tc.tile_pool(name="rinvp", bufs=4) as rinvp,
        ):
            qT_t = [consts.tile([128, L], bf, tag=f"qT{p}") for p in range(NPAIR)]
            kT_t = [consts.tile([128, L], bf, tag=f"kT{p}") for p in range(NPAIR)]
            vo_t = [consts.tile([128, NCHUNK * 65], bf, tag=f"vo{h}")
                    for h in range(HPC)]
            nc.sync.dma_start(out=kT_t[0], in_=kT[0])
            nc.sync.dma_start(out=qT_t[0], in_=qT[0])
            for h in (0, 1):
                nc.gpsimd.dma_start(out=vo_t[h], in_=vo[h])
            for p in (1, 2, 3):
                nc.sync.dma_start(out=kT_t[p], in_=kT[p])
                nc.sync.dma_start(out=qT_t[p], in_=qT[p])
            for h in range(2, HPC):
                nc.gpsimd.dma_start(out=vo_t[h], in_=vo[h])
            msk_t = []
            for c in range(NCHUNK):
                t = consts.tile([128, L], bf, tag=f"msk{c}")
                nc.gpsimd.dma_start(out=t, in_=msk[c])
                msk_t.append(t)

            pmaps = {}
            # simple per-chunk pipeline (correctness-focused fallback)
            for p in range(NPAIR):
                for hl, rows in ((0, slice(0, 64)), (1, slice(64, 128))):
                    h = 2 * p + hl
                    pmaps[h] = [None] * NCHUNK
                    for ci in range(NCHUNK):
                        ps = psumS.tile([128, 1024], f32, tag="psS", name="psS")
                        for s0 in range(0, L, 512):
                            nc.tensor.matmul(
                                ps[:, s0:s0 + 512],
                                lhsT=kT_t[p][rows, 128 * ci:128 * ci + 128],
                                rhs=qT_t[p][rows, s0:s0 + 512],
                                start=True, stop=True,
                            )
                        pt = pTpool.tile([128, 1024], bf, tag="pt", name="pt")
                        nc.scalar.activation(
                            pt, ps, mybir.ActivationFunctionType.Exp,
                            scale=SCALE)
                        nc.vector.tensor_mul(pt, pt, msk_t[ci])
                        pmaps[h][ci] = pt
                    obuf = outsb.tile([128, 512], f32, tag=f"ob{h % 4}",
                                      name=f"ob{h}")
                    for half in (0, 1):
                        po = psumO.tile([128, 325], f32, tag="psO", name="psO")
                        for bi in range(4):
                            j, coff = 4 * half + bi, 65 * bi
                            for idx, i2 in enumerate(range(NCHUNK)):
                                pt = pmaps[h][i2]
                                nc.tensor.matmul(
                                    po[:, coff:coff + 65],
                                    lhsT=pt[:, 128 * j:128 * j + 128],
                                    rhs=vo_t[h][:, 65 * i2:65 * i2 + 65],
                                    start=(idx == 0), stop=(idx == NCHUNK - 1),
                                )
                        rinv4 = rinvp.tile([128, 4], f32, tag="rinv", name="rinv")
                        poap = po[:, :]
                        rsrc = bass.AP(tensor=poap.tensor,
                                       offset=poap.offset + 64,
                                       ap=[poap.ap[0], [65, 4]])
                        nc.vector.reciprocal(rinv4, rsrc)
                        o_in = bass.AP(tensor=poap.tensor, offset=poap.offset,
                                       ap=[poap.ap[0], [65, 4], [1, 64]])
                        rap = rinv4[:, :]
                        r_in = bass.AP(tensor=rap.tensor, offset=rap.offset,
                                       ap=[rap.ap[0], [1, 4], [0, 64]])
                        oap = obuf[:, :]
                        o_out = bass.AP(tensor=oap.tensor,
                                        offset=oap.offset + 256 * half,
                                        ap=[oap.ap[0], [64, 4], [1, 64]])
                        nc.vector.tensor_mul(o_out, o_in, r_in)
                    nc.sync.dma_start(out=out[h], in_=obuf)
    nc.compile()
    return nc


_CACHE = {}


def _get_nc(causal: bool):
    if causal not in _CACHE:
        _CACHE[causal] = _build_causal() if causal else _build_noncausal()
    return _CACHE[causal]


def kernel(queries, keys, values, attn_mask):
    global LAST_RESULTS
    q = np.asarray(queries).reshape(HEADS, L, E)
    k = np.asarray(keys).reshape(HEADS, L, E)
    v = np.asarray(values).reshape(HEADS, L, E)
    mask = np.asarray(attn_mask).reshape(L, L)
    causal = bool(np.array_equal(mask, np.triu(np.ones((L, L), bool), k=1)))

    nc = _get_nc(causal)

    tri = np.triu(np.ones((128, 128), np.float32), k=0).astype(bfloat16)
    if not causal:
        m01 = np.where(mask, 0.0, 1.0).astype(np.float32).T
        msk = np.ascontiguousarray(m01).reshape(NCHUNK, 128, L).astype(bfloat16)

    in_maps = []
    for c in range(N_CORES):
        hs = slice(c * HPC, (c + 1) * HPC)
        qTm = np.ascontiguousarray(
            q[hs].transpose(0, 2, 1)).astype(bfloat16).reshape(NPAIR, 128, L)
        kTm = np.ascontiguousarray(
            k[hs].transpose(0, 2, 1)).astype(bfloat16).reshape(NPAIR, 128, L)
        vh = v[hs].astype(np.float32)
        vcat = np.concatenate(
            [vh, np.ones((HPC, L, 1), np.float32)], axis=2)  # [8, L, 65]
        vom = np.ascontiguousarray(
            vcat.reshape(HPC, NCHUNK, 128, 65).transpose(0, 2, 1, 3)
        ).astype(bfloat16).reshape(HPC, 128, NCHUNK * 65)
        if causal:
            # split layout: [K^T c0 | Q^T 0:512 | Q^T 512:1024 | K^T c1..c7]
            qkm = np.ascontiguousarray(np.concatenate(
                [kTm[:, :, 0:128], qTm[:, :, 0:512],
                 qTm[:, :, 512:1024], kTm[:, :, 128:1024]],
                axis=2))                                      # [NPAIR,128,2048]
            vop = np.ascontiguousarray(
                vom.reshape(NPAIR, 2, 128, NCHUNK * 65).transpose(0, 2, 1, 3)
            ).reshape(NPAIR, 128, 2 * NCHUNK * 65)
            im = {"qk": qkm, "vo": vop, "tri": tri}
        else:
            im = {"qT": qTm, "kT": kTm, "vo": vom, "msk": msk}
        in_maps.append(im)

    trace = bool(os.environ.get("BASS_ATTN_TRACE"))
    res = run_bass_kernel_spmd(nc, in_maps, core_ids=list(range(N_CORES)),
                               trace=trace)
    LAST_RESULTS = res
    # out[c]: [HPC, 128, 512] = [h, p, j*64+e]; q = 128*j + p
    outs = np.stack([res.results[c]["out"] for c in range(N_CORES)])
    outs = outs.reshape(N_CORES, HPC, 128, NCHUNK, E).transpose(0, 1, 3, 2, 4)
    return np.ascontiguousarray(
        outs.reshape(B, C, H, L, E)).astype(np.float32)


# revision 44
# speedup vs baseline: 1.0408x; 1.0018x over previous
"""AnomalyAttention (causal multi-head attention) on 8 TRN2 NeuronCores.

Problem: B=2, C=4, H=8, L=1024, E=64  ->  64 independent heads.
Sharding: 8 heads per core (data parallel over B*C*H), no collectives.

Per-core pipeline, software-pipelined at S^T pair-tile granularity: the
causal-valid 4608 S^T columns per head are packed into nine 512-col bins;
a PSUM pair-tile [128, 1024] holds one bin for BOTH heads of a head pair
(separate PSUM banks), 36 tiles per core:
  S^T[k, q] = sum_e K[k, e] Q[q, e]   TensorE, the two heads' 64-row matmuls
                                      run CONCURRENTLY via tile_position row
                                      tiling (measured 2x throughput)
  P^T = exp(0.125 * S^T) -> bf16      ~72% of tiles on ScalarE (exact exp),
                                      ~28% on VectorE via a one-instruction
                                      Schraudolph bitcast approximation
  diagonal 0/1 tri masks              VectorE, strided over both heads
  [O | r][q,:] = sum_k P^T[k,q] [V|1] TensorE, PSUM accumulate
  out[q, e] = O[q, e] / r[q]          VectorE reciprocal + bcast multiply

The exp window (~1.0us) is the pacing unit: during window i the PE fills
pair-tile i+1 (3 PSUM slots ping-pong) and drains a batch of AV matmuls of
the previous head pair. Input DMAs are staged: pair 0 first, pair 1
released by gate ops at early-window time, pairs 2/3 hardware-gated by
reusing pairs 0/1's SBUF tiles (bufs=1 tag sharing). Output is bf16,
upcast on the host.

Inputs are pre-transposed / bf16-cast / padded on the host as part of
sharding. Output DRAM layout [h, q%128, (q//128)*64 + e].
"""

import os
import numpy as np
from ml_dtypes import bfloat16

import concourse.bass as bass
import concourse.tile as tile
from concourse import bacc, mybir
from concourse.bass_utils import run_bass_kernel_spmd

B, C, H, L, E = 2, 4, 8, 1024, 64
N_CORES = 8
HEADS = B * C * H              # 64
HPC = HEADS // N_CORES         # 8 heads per core
NPAIR = HPC // 2               # 4 head pairs per core
NCHUNK = L // 128              # 8 k-chunks of 128
SCALE = 1.0 / 8.0

# causal S^T packing: per head, the 4608 valid columns (chunk i has width
# 1024-128i) are packed into nine 512-col bins. A PSUM pair-tile [128, 1024]
# holds one bin for BOTH heads of a pair: head 0 at cols 0:512 (bank 0),
# head 1 at 512:1024 (bank 1) — bank-pure so the two heads' S matmuls can run
# CONCURRENTLY via tile_position row tiling without sharing a PSUM bank.
# Each bin: list of (chunk, col_start_in_chunk, width, col_offset_in_bin).
S_BINS = [
    [(0, 0, 512, 0)],
    [(0, 512, 512, 0)],
    [(1, 0, 512, 0)],
    [(1, 512, 384, 0), (3, 0, 128, 384)],
    [(3, 128, 512, 0)],
    [(2, 0, 512, 0)],
    [(2, 512, 256, 0), (6, 0, 256, 256)],
    [(4, 0, 512, 0)],
    [(5, 0, 384, 0), (7, 0, 128, 384)],
]
# bin -> col offsets of 128-wide diagonal blocks needing the 0/1 tri mask
MASK_OFFS = {0: [0], 2: [0], 3: [384], 5: [0], 6: [256], 7: [0], 8: [0, 384]}

# bins whose exp runs on VectorE via the Schraudolph bitcast trick (one
# tensor_scalar: bf16_bits = int16(x * SCHRAUD_A + SCHRAUD_B)); 10/36 bins
# keeps the extra relative error ~0.6% while offloading ~28% of the exp
# work from ScalarE.  Single-diagonal tri masks run on GpSimd (otherwise
# idle), the rest on VectorE.
DVE_BINS = frozenset({1, 4})
DVE_BINS_EXTRA = frozenset({7})        # on pairs 0 and 1 only
GPS_MASK_BINS = frozenset(
    int(x) for x in os.environ.get("ATTN_GPS_MASKS", "").split(",") if x)
SCHRAUD_A = SCALE * np.log2(np.e) * 128.0
SCHRAUD_B = 127.0 * 128.0 - 6.25

# SBUF layout of the per-pair qk data, three tiles per pair so transfers can
# be released progressively (dram col order: k_c0 | q | k_c1..c7):
#   tile Aa [128, 640]: cols 0:128 = K^T chunk 0; 128:640 = Q^T q 0:512
#   tile Ab [128, 512]: Q^T q 512:1024
#   tile B  [128, 896]: K^T chunks 1..7
QAA_W, QAB_W, QKB_W = 640, 512, 896


def _q_segs(qlo, w):
    """Split q range [qlo, qlo+w) at 512 -> list of (tile_idx, col, width).
    tile_idx: 0 = Aa (col 128+q), 1 = Ab (col q-512)."""
    segs = []
    if qlo < 512:
        wa = min(w, 512 - qlo)
        segs.append((0, 128 + qlo, wa))
        qlo, w = qlo + wa, w - wa
    if w > 0:
        segs.append((1, qlo - 512, w))
    return segs

WARM_MMS = int(os.environ.get("ATTN_WARM_MMS", "17"))
KEEPALIVE = int(os.environ.get("ATTN_KEEPALIVE", "1"))
AV_BATCH = int(os.environ.get("ATTN_AV_BATCH", "8"))
DRAIN_RAMP = int(os.environ.get("ATTN_DRAIN_RAMP", "0"))

LAST_RESULTS = None


def _bank_splits(c0, c1):
    """Split tile-column range [c0, c1) at 512-aligned boundaries."""
    cuts = [c0]
    nb = (c0 // 512 + 1) * 512
    while nb < c1:
        cuts.append(nb)
        nb += 512
    cuts.append(c1)
    return list(zip(cuts[:-1], cuts[1:]))


def _build_causal():
    nc = bacc.Bacc("TRN2", target_bir_lowering=False, debug=False,
                   num_devices=N_CORES)
    bf = mybir.dt.bfloat16
    f32 = mybir.dt.float32

    # qk[p]: [128, 2048] = [K^T pair-stack | Q^T pair-stack]
    qk = nc.dram_tensor("qk", [NPAIR, 128, 2 * L], bf, kind="ExternalInput").ap()
    # vo[p]: [128, 1040] = head 2p cols 0:520, head 2p+1 cols 520:1040
    vo = nc.dram_tensor("vo", [NPAIR, 128, 2 * NCHUNK * 65], bf,
                        kind="ExternalInput").ap()
    tri = nc.dram_tensor("tri", [128, 128], bf, kind="ExternalInput").ap()
    out = nc.dram_tensor("out", [HPC, 128, 512], bf, kind="ExternalOutput").ap()

    with tile.TileContext(nc) as tc:
        with (
            tc.tile_pool(name="consts", bufs=1) as consts,
            tc.tile_pool(name="pT", bufs=20) as pTpool,
            tc.tile_pool(name="psumS", bufs=3, space="PSUM") as psumS,
            tc.tile_pool(name="psumO", bufs=2, space="PSUM") as psumO,
            tc.tile_pool(name="outsb", bufs=2) as outsb,
            tc.tile_pool(name="rinvp", bufs=4) as rinvp,
        ):
            # ---- ACT exp-table warm (emitted first so the table load
            # is off the critical path) ----
            warm = consts.tile([128, 8], f32, tag="warm")
            nc.vector.memset(warm, 0.0)
            warm2 = consts.tile([128, 8], f32, tag="warm2")
            nc.scalar.activation(warm2, warm, mybir.ActivationFunctionType.Exp)
            wsrc = consts.tile([128, 260], bf, tag="wsrc")
            nc.vector.memset(wsrc, 0.0)

            # ---- staged input DMAs ----
            # Three qk tiles per pair; pairs 2/3 reuse pairs 0/1's SBUF tiles
            # (bufs=1 tag sharing), so their transfers are hardware-gated
            # until the earlier pair's last matmul read. Pair 1's transfers
            # are gated on tiny reader ops that execute at early-window time
            # on the vector queue, so pair 0's transfers get the DMA
            # bandwidth to themselves.
            qkAa_t = [consts.tile([128, QAA_W], bf, tag=f"qkAa{p % 2}",
                                  name=f"qkAa{p}") for p in range(NPAIR)]
            qkAb_t = [consts.tile([128, QAB_W], bf, tag=f"qkAb{p % 2}",
                                  name=f"qkAb{p}") for p in range(NPAIR)]
            qkB_t = [consts.tile([128, QKB_W], bf, tag=f"qkB{p % 2}",
                                 name=f"qkB{p}") for p in range(NPAIR)]
            vo_t = [consts.tile([128, 2 * NCHUNK * 65], bf, tag=f"vo{p % 2}",
                                name=f"vot{p}")
                    for p in range(NPAIR)]
            tri_t = consts.tile([128, 128], bf, tag="tri")
            gscr = consts.tile([128, 2], f32, tag="gscr")
            # init corners of the gated tiles (read by the gate ops)
            for t in (qkAa_t[1], qkAb_t[1], qkB_t[1], vo_t[1], vo_t[0]):
                nc.vector.memset(t[0:1, 0:2], 0.0)
            nc.sync.dma_start(out=qkAa_t[0], in_=qk[0][:, 0:QAA_W])
            nc.sync.dma_start(out=qkAb_t[0], in_=qk[0][:, QAA_W:QAA_W + QAB_W])
            nc.scalar.dma_start(out=qkB_t[0], in_=qk[0][:, QAA_W + QAB_W:2 * L])
            nc.gpsimd.dma_start(out=vo_t[0], in_=vo[0])
            nc.sync.dma_start(out=tri_t, in_=tri)
            # pairs 2/3 (naturally gated by tag reuse)
            for p in (2, 3):
                nc.sync.dma_start(out=qkAa_t[p], in_=qk[p][:, 0:QAA_W])
                nc.sync.dma_start(out=qkAb_t[p],
                                  in_=qk[p][:, QAA_W:QAA_W + QAB_W])
                nc.sync.dma_start(out=qkB_t[p],
                                  in_=qk[p][:, QAA_W + QAB_W:2 * L])
                nc.gpsimd.dma_start(out=vo_t[p], in_=vo[p])

            last_pts = {}

            def _gate(tile_t, ready_t):
                # reader op on the (idle) gpsimd queue: finishes only once
                # ready_t's DMA has landed, releasing the DMA emitted after it
                nc.gpsimd.tensor_mul(gscr[0:1, 0:2], tile_t[0:1, 0:2],
                                     ready_t[0:1, 0:2])

            def deferred_dma(i):
                if i == 1:
                    # release pair-1's inputs once ALL of pair-0's qk data
                    # (incl. the scalar-ring k_c1..c7 transfer) has landed
                    _gate(qkAa_t[1], qkB_t[0])
                    nc.sync.dma_start(out=qkAa_t[1], in_=qk[1][:, 0:QAA_W])
                    _gate(qkAb_t[1], qkB_t[0])
                    nc.sync.dma_start(out=qkAb_t[1],
                                      in_=qk[1][:, QAA_W:QAA_W + QAB_W])
                elif i == 4:
                    _gate(qkB_t[1], qkB_t[0])
                    nc.scalar.dma_start(out=qkB_t[1],
                                        in_=qk[1][:, QAA_W + QAB_W:2 * L])
                elif i == 7:
                    _gate(vo_t[1], vo_t[0])
                    nc.gpsimd.dma_start(out=vo_t[1], in_=vo[1])

            def pe_warm(n):
                for _ in range(n):
                    wps = psumO.tile([128, 325], f32, tag="psO", name="wps")
                    nc.tensor.matmul(wps[:, 0:260], lhsT=wsrc[:, 0:128],
                                     rhs=wsrc, start=True, stop=True)
            pe_warm(WARM_MMS)

            # ---- tile list: window order (pair, bin) ----
            tiles = [(p, b) for p in range(NPAIR) for b in range(len(S_BINS))]

            pmaps = {}          # head -> {(chunk, block): (pt, col)}
            obufs = {}
            done_halves = {}

            q_tiles = [qkAa_t, qkAb_t]

            def emit_fill(p, b):
                ps = psumS.tile([128, 1024], f32, tag="psS", name="psS")
                # the two heads' matmuls are emitted adjacent with explicit
                # row tile_position so they execute concurrently; each head
                # writes its own PSUM bank (cols 0:512 / 512:1024)
                for (ci, cc0, w, off) in S_BINS[b]:
                    if ci == 0:
                        klhs = qkAa_t[p]
                        kc = 0
                    else:
                        klhs = qkB_t[p]
                        kc = 128 * (ci - 1)
                    for (qt, qc, sw) in _q_segs(128 * ci + cc0, w):
                        for hl in (0, 1):
                            rows = slice(64 * hl, 64 * hl + 64)
                            nc.tensor.matmul(
                                ps[:, 512 * hl + off:512 * hl + off + sw],
                                lhsT=klhs[rows, kc:kc + 128],
                                rhs=q_tiles[qt][p][rows, qc:qc + sw],
                                start=True, stop=True,
                                tile_position=(64 * hl, 0),
                            )
                        off += sw
                return ps

            def emit_exp(p, b, ps):
                pt = pTpool.tile([128, 1024], bf, tag="pt", name="pt")
                if b in DVE_BINS or (b in DVE_BINS_EXTRA and p < 2):
                    # fast exp on VectorE: bf16 bit pattern built directly by
                    # an affine op + float->int16 convert (Schraudolph)
                    pti = pt[:, :].bitcast(mybir.dt.int16)
                    nc.vector.tensor_scalar(
                        pti, ps, SCHRAUD_A, SCHRAUD_B,
                        mybir.AluOpType.mult, mybir.AluOpType.add)
                else:
                    nc.scalar.activation(
                        pt, ps, mybir.ActivationFunctionType.Exp, scale=SCALE)
                for hl in (0, 1):
                    h = 2 * p + hl
                    pm = pmaps.setdefault(h, {})
                    for (ci, cc0, w, off) in S_BINS[b]:
                        for blk in range(w // 128):
                            pm[(ci, cc0 // 128 + blk)] = (
                                pt, 512 * hl + off + 128 * blk)
                return pt

            def emit_mask(b, pt):
                offs = MASK_OFFS.get(b)
                if not offs:
                    return
                ptap = pt[:, :]
                tap = tri_t[:, :]
                if len(offs) == 1:
                    free = [[512, 2], [1, 128]]
                    tfree = [[0, 2], [1, 128]]
                else:
                    free = [[512, 2], [offs[1] - offs[0], 2], [1, 128]]
                    tfree = [[0, 2], [0, 2], [1, 128]]
                src = bass.AP(tensor=ptap.tensor, offset=ptap.offset + offs[0],
                              ap=[ptap.ap[0]] + free)
                trib = bass.AP(tensor=tap.tensor, offset=tap.offset,
                               ap=[tap.ap[0]] + tfree)
                eng = nc.gpsimd if b in GPS_MASK_BINS else nc.vector
                eng.tensor_mul(src, src, trib)

            # ---- AV op queue ----
            # items: ("mm", fn) costing 1, ("aux", fn) costing 0
            av_q = []
            cur_po = [None]     # live psumO tile for keepalives

            def make_unit_ops(p, h, half, last_pair):
                hl = h - 2 * p
                ops = []

                def alloc_po():
                    po = psumO.tile([128, 325], f32, tag="psO", name="psO")
                    cur_po[0] = po
                    if h not in obufs:
                        obufs[h] = outsb.tile([128, 512], bf,
                                              tag=f"ob{h % 4}", name=f"ob{h}")
                    return po

                state = {}

                def first_mm():
                    state["po"] = alloc_po()

                for bi in range(4):
                    j, coff = 4 * half + bi, 65 * bi
                    ks = list(range(j + 1))
                    for idx, i2 in enumerate(ks):
                        def mm(bi=bi, j=j, coff=coff, idx=idx, i2=i2,
                               first=(bi == 0 and idx == 0)):
                            if first:
                                first_mm()
                            po = state["po"]
                            pt, cs = pmaps[h][(i2, j - i2)]
                            nc.tensor.matmul(
                                po[:, coff:coff + 65],
                                lhsT=pt[:, cs:cs + 128],
                                rhs=vo_t[p][:, 520 * hl + 65 * i2:
                                            520 * hl + 65 * i2 + 65],
                                start=(idx == 0), stop=(idx == len(ks) - 1),
                            )
                        ops.append(("mm", mm))

                def aux():
                    po = state["po"]
                    obuf = obufs[h]
                    rinv4 = rinvp.tile([128, 4], mybir.dt.float32,
                                       tag="rinv", name="rinv")
                    poap = po[:, :]
                    rsrc = bass.AP(tensor=poap.tensor, offset=poap.offset + 64,
                                   ap=[poap.ap[0], [65, 4]])
                    nc.vector.reciprocal(rinv4, rsrc)
                    o_in = bass.AP(tensor=poap.tensor, offset=poap.offset,
                                   ap=[poap.ap[0], [65, 4], [1, 64]])
                    rap = rinv4[:, :]
                    r_in = bass.AP(tensor=rap.tensor, offset=rap.offset,
                                   ap=[rap.ap[0], [1, 4], [0, 64]])
                    oap = obuf[:, :]
                    o_out = bass.AP(tensor=oap.tensor,
                                    offset=oap.offset + 256 * half,
                                    ap=[oap.ap[0], [64, 4], [1, 64]])
                    nc.vector.tensor_mul(o_out, o_in, r_in)
                    if cur_po[0] is po:
                        cur_po[0] = None
                    done_halves.setdefault(h, set()).add(half)
                    if last_pair:
                        # stream each finished 256-col slab out immediately;
                        # scalar queue is idle after the last exp while sync
                        # is backlogged with earlier triggers
                        nc.scalar.dma_start(
                            out=out[h][:, 256 * half:256 * half + 256],
                            in_=obuf[:, 256 * half:256 * half + 256])
                        if len(done_halves[h]) == 2:
                            obufs.pop(h)
                    elif len(done_halves[h]) == 2:
                        nc.sync.dma_start(out=out[h], in_=obufs.pop(h))
                ops.append(("aux", aux))
                return ops

            def append_pair_av(p, last_pair):
                order = [(2 * p, 0), (2 * p + 1, 0), (2 * p, 1), (2 * p + 1, 1)]
                for h, half in order:
                    av_q.extend(make_unit_ops(p, h, half, last_pair))

            def keepalive():
                po = cur_po[0]
                if KEEPALIVE and po is not None:
                    # accumulate 0 into scratch cols; start=False so the
                    # bank's live has_written state is untouched
                    nc.tensor.matmul(po[:, 260:325], lhsT=wsrc[:, 0:128],
                                     rhs=wsrc[:, 0:65], start=False, stop=False,
                                     skip_group_check=True)

            def drain(n_mms):
                got = 0
                since_ka = 0
                while av_q and got < n_mms:
                    kind, fn = av_q.pop(0)
                    fn()
                    if kind == "mm":
                        got += 1
                        since_ka += 1
                        if since_ka >= 6:
                            keepalive()
                            since_ka = 0
                return got

            # ---- main pipeline ----
            nb = len(S_BINS)
            pl = NPAIR - 1
            for i, (p, b) in enumerate(tiles):
                deferred_dma(i)
                if p >= 1 and (i == p * nb):
                    append_pair_av(p - 1, last_pair=False)
                if i == (NPAIR - 1) * nb + 7:
                    # last pair: half-0 AV needs only chunks 0..3 (bins 0..6,
                    # all emitted) — overlap it with the final two windows
                    for h in (2 * pl, 2 * pl + 1):
                        av_q.extend(make_unit_ops(pl, h, 0, last_pair=True))
                ps = emit_fill(p, b)
                pt = emit_exp(p, b, ps)
                last_pts[i] = pt
                emit_mask(b, pt)
                if i < (NPAIR - 1) * nb + 7:
                    # lighter AV batches early in each pair (fills + window
                    # transitions are busiest), heavier late
                    budget = AV_BATCH + (b // 3 - 1) * 2 if DRAIN_RAMP else \
                        AV_BATCH
                else:
                    budget = AV_BATCH + 6
                drain(budget)
            # only the last pair's half-1 AV remains after the final exp
            for h, half in [(2 * pl, 1), (2 * pl + 1, 1)]:
                av_q.extend(make_unit_ops(pl, h, half, last_pair=True))
            while av_q:
                kind, fn = av_q.pop(0)
                fn()
    nc.compile()
    return nc


def _build_noncausal():
    """Fallback path (reference mask that is not the causal triangle)."""
    nc = bacc.Bacc("TRN2", target_bir_lowering=False, debug=False,
                   num_devices=N_CORES)
    bf = mybir.dt.bfloat16
    f32 = mybir.dt.float32

    qT = nc.dram_tensor("qT", [NPAIR, 128, L], bf, kind="ExternalInput").ap()
    kT = nc.dram_tensor("kT", [NPAIR, 128, L], bf, kind="ExternalInput").ap()
    vo = nc.dram_tensor("vo", [HPC, 128, NCHUNK * 65], bf, kind="ExternalInput").ap()
    msk = nc.dram_tensor("msk", [NCHUNK, 128, L], bf, kind="ExternalInput").ap()
    out = nc.dram_tensor("out", [HPC, 128, 512], bf, kind="ExternalOutput").ap()

    with tile.TileContext(nc) as tc:
        with (
            tc.tile_pool(name="consts", bufs=1) as consts,
            tc.tile_pool(name="pT", bufs=32) as pTpool,
            tc.tile_pool(name="psumS", bufs=2, space="PSUM") as psumS,
            tc.tile_pool(name="psumO", bufs=2, space="PSUM") as psumO,
            tc.tile_pool(name="outsb", bufs=2) as outsb,
            tc.tile_pool(name="rinvp", bufs=4) as rinvp,
        ):
            qT_t = [consts.tile([128, L], bf, tag=f"qT{p}") for p in range(NPAIR)]
            kT_t = [consts.tile([128, L], bf, tag=f"kT{p}") for p in range(NPAIR)]
            vo_t = [consts.tile([128, NCHUNK * 65], bf, tag=f"vo{h}")
                    for h in range(HPC)]
            nc.sync.dma_start(out=kT_t[0], in_=kT[0])
            nc.sync.dma_start(out=qT_t[0], in_=qT[0])
            for h in (0, 1):
                nc.gpsimd.dma_start(out=vo_t[h], in_=vo[h])
            for p in (1, 2, 3):
                nc.sync.dma_start(out=kT_t[p], in_=kT[p])
                nc.sync.dma_start(out=qT_t[p], in_=qT[p])
            for h in range(2, HPC):
                nc.gpsimd.dma_start(out=vo_t[h], in_=vo[h])
            msk_t = []
            for c in range(NCHUNK):
                t = consts.tile([128, L], bf, tag=f"msk{c}")
                nc.gpsimd.dma_start(out=t, in_=msk[c])
                msk_t.append(t)

            pmaps = {}
            # simple per-chunk pipeline (correctness-focused fallback)
            for p in range(NPAIR):
                for hl, rows in ((0, slice(0, 64)), (1, slice(64, 128))):
                    h = 2 * p + hl
                    pmaps[h] = [None] * NCHUNK
                    for ci in range(NCHUNK):
                        ps = psumS.tile([128, 1024], f32, tag="psS", name="psS")
                        for s0 in range(0, L, 512):
                            nc.tensor.matmul(
                                ps[:, s0:s0 + 512],
                                lhsT=kT_t[p][rows, 128 * ci:128 * ci + 128],
                                rhs=qT_t[p][rows, s0:s0 + 512],
                                start=True, stop=True,
                            )
                        pt = pTpool.tile([128, 1024], bf, tag="pt", name="pt")
                        nc.scalar.activation(
                            pt, ps, mybir.ActivationFunctionType.Exp,
                            scale=SCALE)
                        nc.vector.tensor_mul(pt, pt, msk_t[ci])
                        pmaps[h][ci] = pt
                    obuf = outsb.tile([128, 512], f32, tag=f"ob{h % 4}",
                                      name=f"ob{h}")
                    for half in (0, 1):
                        po = psumO.tile([128, 325], f32, tag="psO", name="psO")
                        for bi in range(4):
                            j, coff = 4 * half + bi, 65 * bi
                            for idx, i2 in enumerate(range(NCHUNK)):
                                pt = pmaps[h][i2]
                                nc.tensor.matmul(
                                    po[:, coff:coff + 65],
                                    lhsT=pt[:, 128 * j:128 * j + 128],
                                    rhs=vo_t[h][:, 65 * i2:65 * i2 + 65],
                                    start=(idx == 0), stop=(idx == NCHUNK - 1),
                                )
                        rinv4 = rinvp.tile([128, 4], f32, tag="rinv", name="rinv")
                        poap = po[:, :]
                        rsrc = bass.AP(tensor=poap.tensor,
                                       offset=poap.offset + 64,
                                       ap=[poap.ap[0], [65, 4]])
                        nc.vector.reciprocal(rinv4, rsrc)
                        o_in = bass.AP(tensor=poap.tensor, offset=poap.offset,
                                       ap=[poap.ap[0], [65, 4], [1, 64]])
                        rap = rinv4[:, :]
                        r_in = bass.AP(tensor=rap.tensor, offset=rap.offset,
                                       ap=[rap.ap[0], [1, 4], [0, 64]])
                        oap = obuf[:, :]
                        o_out = bass.AP(tensor=oap.tensor,
                                        offset=oap.offset + 256 * half,
                                        ap=[oap.ap[0], [64, 4], [1, 64]])
                        nc.vector.tensor_mul(o_out, o_in, r_in)
                    nc.sync.dma_start(out=out[h], in_=obuf)
    nc.compile()
    return nc


_CACHE = {}


def _get_nc(causal: bool):
    if causal not in _CACHE:
        _CACHE[causal] = _build_causal() if causal else _build_noncausal()
    return _CACHE[causal]


def kernel(queries, keys, values, attn_mask):
    global LAST_RESULTS
    q = np.asarray(queries).reshape(HEADS, L, E)
    k = np.asarray(keys).reshape(HEADS, L, E)
    v = np.asarray(values).reshape(HEADS, L, E)
    mask = np.asarray(attn_mask).reshape(L, L)
    causal = bool(np.array_equal(mask, np.triu(np.ones((L, L), bool), k=1)))

    nc = _get_nc(causal)

    tri = np.triu(np.ones((128, 128), np.float32), k=0).astype(bfloat16)
    if not causal:
        m01 = np.where(mask, 0.0, 1.0).astype(np.float32).T
        msk = np.ascontiguousarray(m01).reshape(NCHUNK, 128, L).astype(bfloat16)

    in_maps = []
    for c in range(N_CORES):
        hs = slice(c * HPC, (c + 1) * HPC)
        qTm = np.ascontiguousarray(
            q[hs].transpose(0, 2, 1)).astype(bfloat16).reshape(NPAIR, 128, L)
        kTm = np.ascontiguousarray(
            k[hs].transpose(0, 2, 1)).astype(bfloat16).reshape(NPAIR, 128, L)
        vh = v[hs].astype(np.float32)
        vcat = np.concatenate(
            [vh, np.ones((HPC, L, 1), np.float32)], axis=2)  # [8, L, 65]
        vom = np.ascontiguousarray(
            vcat.reshape(HPC, NCHUNK, 128, 65).transpose(0, 2, 1, 3)
        ).astype(bfloat16).reshape(HPC, 128, NCHUNK * 65)
        if causal:
            # split layout: [K^T c0 | Q^T 0:512 | Q^T 512:1024 | K^T c1..c7]
            qkm = np.ascontiguousarray(np.concatenate(
                [kTm[:, :, 0:128], qTm[:, :, 0:512],
                 qTm[:, :, 512:1024], kTm[:, :, 128:1024]],
                axis=2))                                      # [NPAIR,128,2048]
            vop = np.ascontiguousarray(
                vom.reshape(NPAIR, 2, 128, NCHUNK * 65).transpose(0, 2, 1, 3)
            ).reshape(NPAIR, 128, 2 * NCHUNK * 65)
            im = {"qk": qkm, "vo": vop, "tri": tri}
        else:
            im = {"qT": qTm, "kT": kTm, "vo": vom, "msk": msk}
        in_maps.append(im)

    trace = bool(os.environ.get("BASS_ATTN_TRACE"))
    res = run_bass_kernel_spmd(nc, in_maps, core_ids=list(range(N_CORES)),
                               trace=trace)
    LAST_RESULTS = res
    # out[c]: [HPC, 128, 512] = [h, p, j*64+e]; q = 128*j + p
    outs = np.stack([res.results[c]["out"] for c in range(N_CORES)])
    outs = outs.reshape(N_CORES, HPC, 128, NCHUNK, E).transpose(0, 1, 3, 2, 4)
    return np.ascontiguousarray(
        outs.reshape(B, C, H, L, E)).astype(np.float32)
